# revision 22
# baseline (speedup 1.0000x reference)
# Dilated causal self-attention kernel for Trainium2 (8 NeuronCores).
#
# Reference computation (see problem):
#   x (4, 8192, 1024) -> reshape (4, 4, 2048, 1024) -> take every 4th token
#   -> per-segment causal MHA (16 heads, dh=64) -> scatter back into zeros.
#
# Sharding: 16 independent (batch, segment) attention problems, 2 per core.
# Host does the dilated gather + transpose + bf16 cast and the final scatter
# into the zero background; each core runs QKV -> per-head causal softmax
# attention -> output projection on its 2 segments.
#
# Device layout (all feature-major where possible):
#   xiT    [C, M]  (per segment)         - input, bf16
#   qkT    [2C, M] feature-major         - q rows pre-scaled by 1/sqrt(dh)
#                                          (folded into w_in on host)
#   v      [M, C]  token-major           - v bias folds into output bias
#   scores [128 q, n k] per (head, q-chunk), n = (qc+1)*128 (causal skip)
#   p = exp(scores) (no max subtraction: scores ~ N(0,1)), accum_out = denom
#   PV: outT[dh, M] = sum_kc v_kc^T @ pT_kc   (pT via PE transpose)
#   yT = w_outT^T @ oT + b_out_eff  -> DMA out feature-major (bf16)
#
# Schedule notes: weights packed e-tile-major so each consumer's slice is
# one contiguous DMA, with the first x/weight pieces split fine so the PE
# chases the DMA ramp (~11us to first matmul); seg1's projection is split
# into a ct0-3 half run as attention filler and a ct4-7 half fused with the
# bias add at the tail (borrowing the idle score PSUM banks); oT pool is
# deep enough that seg1 norms never wait on proj0 reads; norm chains are
# priority-hoisted; outputs are bf16 and drain on two DMA queues.

import sys

sys.path.insert(0, "/opt/trn_rl_repo")

import numpy as np
import ml_dtypes

import concourse.bacc as bacc
import concourse.mybir as mybir
from concourse.tile import TileContext
from concourse.bass_utils import run_bass_kernel_spmd

BF16 = ml_dtypes.bfloat16

B, N, C = 4, 8192, 1024
W_SEG, RATE, H = 2048, 4, 16
DH = C // H            # 64
S = N // W_SEG         # 4 segments per batch
M = W_SEG // RATE      # 512 tokens per segment
N_CORES = 8
SEG_PER_CORE = (B * S) // N_CORES  # 2

FP32 = mybir.dt.float32
BF = mybir.dt.bfloat16

_CACHE = {}


def _build():
    nc = bacc.Bacc()
    phase_of = _CACHE.setdefault("phase_of", {})

    def mm(phase, *args, **kwargs):
        inst = nc.tensor.matmul(*args, **kwargs)
        try:
            phase_of[inst.ins.name] = phase
        except Exception:
            pass
        return inst

    # e-tile-major packed layouts (each consumer slice = contiguous cols)
    xiT = nc.dram_tensor("xiT", [SEG_PER_CORE * 128, 8 * M], BF, kind="ExternalInput")
    wqk = nc.dram_tensor("wqk", [4 * 128, 4 * 1024], BF, kind="ExternalInput")
    wv = nc.dram_tensor("wv", [128, 8 * C], BF, kind="ExternalInput")
    wout = nc.dram_tensor("wout", [128, 8 * C], BF, kind="ExternalInput")
    bqk = nc.dram_tensor("bqk", [128, 16], FP32, kind="ExternalInput")
    bout = nc.dram_tensor("bout", [128, 8], FP32, kind="ExternalInput")
    yT = nc.dram_tensor("yT", [SEG_PER_CORE * C, M], BF, kind="ExternalOutput")

    CT = C // 128  # 8 contraction chunks

    from contextlib import ExitStack
    with TileContext(nc) as tc, ExitStack() as ctx:
        consts = ctx.enter_context(tc.tile_pool(name="consts", bufs=1))
        wpool = ctx.enter_context(tc.tile_pool(name="weights", bufs=1))
        xpool = ctx.enter_context(tc.tile_pool(name="x", bufs=2))
        qkpool = ctx.enter_context(tc.tile_pool(name="qk", bufs=32))
        vpool = ctx.enter_context(tc.tile_pool(name="v", bufs=8))
        ptpool = ctx.enter_context(tc.tile_pool(name="pt", bufs=12))
        rbpool = ctx.enter_context(tc.tile_pool(name="rb", bufs=4))
        otpool = ctx.enter_context(tc.tile_pool(name="ot", bufs=16))
        h1pool = ctx.enter_context(tc.tile_pool(name="h1", bufs=8))
        ypool = ctx.enter_context(tc.tile_pool(name="y", bufs=8))
        small = ctx.enter_context(tc.tile_pool(name="small", bufs=4))
        psA = ctx.enter_context(tc.tile_pool(name="psA", bufs=2, space="PSUM"))
        psS = ctx.enter_context(tc.tile_pool(name="psS", bufs=4, space="PSUM"))
        psO = ctx.enter_context(tc.tile_pool(name="psO", bufs=2, space="PSUM"))

        if True:
            bqk_sb = consts.tile([128, 16], FP32, tag="bqk")
            bout_sb = consts.tile([128, 8], FP32, tag="bout")

            # wqk_sb[c4][:, off*1024 + ct*128]: off = e-tile within chunk
            # (pp*2 + half); wv[:, nf*4096 + ct*512]; wout[:, ot*1024 + ct*128]
            wqk_sb = [wpool.tile([128, 4 * 1024], BF, tag=f"wqk{c4}", name="w")
                      for c4 in range(4)]
            wv_sb = wpool.tile([128, 8 * C], BF, tag="wv")
            wout_sb = wpool.tile([128, 8 * C], BF, tag="wout")

            # --- input DMAs ---------------------------------------------------
            # all on the sync queue, in first-consumed-first order
            x_sb = {}

            def emit_x(seg, eng):
                t = xpool.tile([128, 8 * M], BF, tag="x", name="x")
                nq = 4 if seg == 0 else 2
                step = 8 * M // nq
                for i in range(nq):
                    eng.dma_start(
                        out=t[:, i * step:(i + 1) * step],
                        in_=xiT[seg * 128:(seg + 1) * 128, i * step:(i + 1) * step])
                x_sb[seg] = t

            # interleave x0 pieces with wqk0 e-tile pieces so the PE can
            # chase the DMA stream: e-tile0's ct-k matmul only needs x piece
            # k/2 and off0, which lands right after x piece 0
            x0 = xpool.tile([128, 8 * M], BF, tag="x", name="x")
            x_sb[0] = x0

            def x0_piece(i):
                nc.sync.dma_start(
                    out=x0[:, i * 1024:(i + 1) * 1024],
                    in_=xiT[0:128, i * 1024:(i + 1) * 1024])

            def wqk0_piece(off):
                nc.sync.dma_start(
                    out=wqk_sb[0][:, off * 1024:(off + 1) * 1024],
                    in_=wqk[0:128, off * 1024:(off + 1) * 1024])

            # extra-fine first pieces: the very first matmul needs only
            # x cols 0:512 and off0 cols 0:512
            nc.sync.dma_start(out=x0[:, 0:512], in_=xiT[0:128, 0:512])
            nc.sync.dma_start(out=wqk_sb[0][:, 0:512], in_=wqk[0:128, 0:512])
            nc.sync.dma_start(out=bqk_sb[:], in_=bqk[:, :])
            nc.sync.dma_start(out=x0[:, 512:1024], in_=xiT[0:128, 512:1024])
            nc.sync.dma_start(out=wqk_sb[0][:, 512:1024], in_=wqk[0:128, 512:1024])
            x0_piece(1)
            wqk0_piece(1)
            x0_piece(2)
            wqk0_piece(2)
            x0_piece(3)
            wqk0_piece(3)
            for c4 in (1, 2, 3):
                nc.sync.dma_start(
                    out=wqk_sb[c4][:], in_=wqk[c4 * 128:(c4 + 1) * 128, :])
            nc.sync.dma_start(out=wv_sb[:], in_=wv[:, :])
            emit_x(1, nc.sync)
            nc.sync.dma_start(out=wout_sb[:], in_=wout[:, :])
            nc.sync.dma_start(out=bout_sb[:], in_=bout[:, :])

            # --- software-pipelined emission ---------------------------------
            # Dense matmul phases (QKV, proj) are interleaved into the
            # attention phase so the PE never idles:
            #   A(0) | B(0)+C(0) with A(1) spread through | B(1)+C(1a)
            #   | C(1b) tail
            qk_sb = {}
            v_sb = {}
            oT_sb = {}

            QK_ORDER = [p + half for p in range(8) for half in (0, 8)]

            def emit_qkv_unit(seg, u):
                # units 0..15: qk e-tiles (interleaved q/k); 16..23: v halves
                if u < 16:
                    et = QK_ORDER[u]
                    p = et % 8
                    c4 = p // 2
                    off = (p % 2) * 2 + (0 if et < 8 else 1)
                    ps = psA.tile([128, M], FP32, tag="psA", name="ps")
                    for ct in range(CT):
                        mm("qkv_qk",
                            ps[:],
                            lhsT=wqk_sb[c4][:, off * 1024 + ct * 128:
                                            off * 1024 + (ct + 1) * 128],
                            rhs=x_sb[seg][:, ct * M:(ct + 1) * M],
                            start=(ct == 0), stop=(ct == CT - 1))
                    t = qkpool.tile([128, M], BF, tag="qk", name="qk")
                    nc.scalar.activation(
                        out=t[:], in_=ps[:],
                        func=mybir.ActivationFunctionType.Identity,
                        bias=bqk_sb[:, et:et + 1], scale=1.0)
                    qk_sb.setdefault(seg, [None] * 16)[et] = t
                else:
                    tt, nf = divmod(u - 16, 2)
                    if nf == 0:
                        vt = vpool.tile([128, 16, 65], BF, tag="v", name="v")
                        v_sb.setdefault(seg, [None] * 4)[tt] = vt
                        # ones column per head: PV row 64 accumulates the
                        # softmax denominator for free
                        nc.vector.memset(vt[:, :, 64:65], 1.0)
                    vt = v_sb[seg][tt]
                    ps = psA.tile([128, M], FP32, tag="psA", name="ps")
                    for ct in range(CT):
                        mm("qkv_v",
                            ps[:],
                            lhsT=x_sb[seg][:, ct * M + tt * 128:ct * M + (tt + 1) * 128],
                            rhs=wv_sb[:, nf * 4096 + ct * 512:nf * 4096 + (ct + 1) * 512],
                            start=(ct == 0), stop=(ct == CT - 1))
                    nc.scalar.copy(
                        out=vt[:, nf * 8:(nf + 1) * 8, 0:64],
                        in_=ps[:].rearrange("p (h e) -> p h e", e=64))

            def emit_scores(seg, h):
                # scoresT blocks [k, q]: lhsT = k-chunk, rhs = q (no
                # transposes needed anywhere; pT = exp(scoresT) directly)
                et, row = h // 2, (h % 2) * 64
                qh = qk_sb[seg][et][row:row + 64, :]
                kh = qk_sb[seg][8 + et][row:row + 64, :]
                pt_sb = []
                for kc in range(4):
                    n2 = (4 - kc) * 128
                    ps = psS.tile([128, M], FP32, tag="psS", name="ps")
                    mm("scores",
                        ps[:, :n2],
                        lhsT=kh[:, kc * 128:(kc + 1) * 128],
                        rhs=qh[:, kc * 128:], start=True, stop=True)
                    ptk = ptpool.tile([128, M], BF, tag="pt", name="pt")
                    nc.scalar.activation(
                        out=ptk[:, :n2], in_=ps[:, :n2],
                        func=mybir.ActivationFunctionType.Exp)
                    # causal mask: zero the lower triangle of the diagonal
                    # block (keep where q_local >= k_local) on idle GpSimd
                    nc.gpsimd.affine_select(
                        out=ptk[:, 0:128], in_=ptk[:, 0:128],
                        compare_op=mybir.AluOpType.is_ge,
                        fill=0.0, base=0,
                        pattern=[[1, 128]], channel_multiplier=-1)
                    pt_sb.append(ptk)
                return pt_sb

            def emit_pv(seg, h, po, pt_sb):
                # po [65, M]: rows 0:64 = unnormalized outT, row 64 = denom
                for kc in range(4):
                    n2 = (4 - kc) * 128
                    mm("pv",
                        po[:, kc * 128:],
                        lhsT=v_sb[seg][kc][:, h, :],
                        rhs=pt_sb[kc][:, :n2],
                        start=(kc == 0), stop=(kc == 3))
                denrow = small.tile([1, M], FP32, tag="denrow", name="denrow", bufs=3)
                # on scalar: vector is the pacing engine in the attn phases
                nc.scalar.copy(out=denrow[:], in_=po[64:65, :])
                rdenT = small.tile([1, M], FP32, tag="rdenT", name="rdenT", bufs=3)
                row = (h % 2) * 64
                # the norm chain gates the projection tiles: let the list
                # scheduler run it ahead of slack work (casts, bias adds)
                with tc.high_priority():
                    nc.vector.reciprocal_approx_fast(out=rdenT[:], in_=denrow[:])
                    rb = rbpool.tile([64, M], FP32, tag="rb", name="rb")
                    nc.gpsimd.partition_broadcast(rb[:], rdenT[:], channels=64)
                    nc.vector.tensor_mul(
                        out=oT_sb[seg][h // 2][row:row + 64, :],
                        in0=po[0:64, :], in1=rb[:])

            def emit_proj_tile(seg, ot):
                # full 8-ct projection tile -> yT (seg0 / attn1 filler)
                base = seg * C
                ps = psA.tile([128, M], FP32, tag="psA", name="ps")
                for ct in range(CT):
                    mm("proj0",
                        ps[:],
                        lhsT=wout_sb[:, ot * 1024 + ct * 128:ot * 1024 + (ct + 1) * 128],
                        rhs=oT_sb[seg][ct][:],
                        start=(ct == 0), stop=(ct == CT - 1))
                yt = ypool.tile([128, M], BF, tag="y", name="yt")
                nc.vector.tensor_scalar_add(yt[:], ps[:], bout_sb[:, ot:ot + 1])
                eng = nc.sync if ot % 2 == 0 else nc.scalar
                eng.dma_start(
                    out=yT[base + ot * 128:base + (ot + 1) * 128, :], in_=yt[:])

            def emit_proj_h1(seg, ot):
                # first half (ct 0-3) of a seg1 projection tile -> sbuf
                ps = psA.tile([128, M], FP32, tag="psA", name="ps")
                for ct in range(4):
                    mm("proj1a",
                        ps[:],
                        lhsT=wout_sb[:, ot * 1024 + ct * 128:ot * 1024 + (ct + 1) * 128],
                        rhs=oT_sb[seg][ct][:],
                        start=(ct == 0), stop=(ct == 3))
                h1 = h1pool.tile([128, M], BF, tag="h1", name="h1")
                nc.vector.tensor_copy(out=h1[:], in_=ps[:])
                return h1

            def emit_proj_h2_mms(seg, ot, cts, ps):
                for ct in cts:
                    mm("proj1b",
                        ps[:],
                        lhsT=wout_sb[:, ot * 1024 + ct * 128:ot * 1024 + (ct + 1) * 128],
                        rhs=oT_sb[seg][ct][:],
                        start=(ct == 4), stop=(ct == CT - 1))

            def emit_proj_h2_fin(seg, ot, ps, h1):
                base = seg * C
                yt = ypool.tile([128, M], BF, tag="y", name="yt")
                nc.vector.scalar_tensor_tensor(
                    out=yt[:], in0=ps[:], scalar=bout_sb[:, ot:ot + 1],
                    in1=h1[:], op0=mybir.AluOpType.add, op1=mybir.AluOpType.add)
                eng = nc.sync if ot % 2 == 0 else nc.scalar
                eng.dma_start(
                    out=yT[base + ot * 128:base + (ot + 1) * 128, :], in_=yt[:])

            def emit_attn(seg, filler, warm=None):
                # two-stage software pipeline over heads: scoresT+exp of head
                # h+1 is emitted before PV(h), covering softmax latency.
                # `warm` carries heads whose scores were pre-emitted into the
                # preceding dense stream (pipeline warm-up).
                oT_sb[seg] = [otpool.tile([128, M], BF, tag="ot", name="ot")
                              for _ in range(8)]
                prev = None
                for h in range(H):
                    if warm and h in warm:
                        cur = (h,) + warm[h]
                    else:
                        cur = (h, emit_scores(seg, h),
                               psO.tile([65, M], FP32, tag="psO", name="po"))
                    if prev is not None:
                        ph, pts, po = prev
                        emit_pv(seg, ph, po, pts)
                    filler(h)
                    prev = cur
                ph, pts, po = prev
                emit_pv(seg, ph, po, pts)

            emit_qkv_unit(0, 0)
            for u in range(1, 20):
                emit_qkv_unit(0, u)
            warm0 = {0: (emit_scores(0, 0),
                         psO.tile([65, M], FP32, tag="psO", name="po"))}
            emit_qkv_unit(0, 20)
            emit_qkv_unit(0, 21)
            warm0[1] = (emit_scores(0, 1),
                        psO.tile([65, M], FP32, tag="psO", name="po"))
            emit_qkv_unit(0, 22)
            emit_qkv_unit(0, 23)

            # B(0) with A(1) spread through: seg1 filler interleaves v into
            # the qk stream (weights long since resident by then)
            A_ORDER = [0, 1, 16, 2, 3, 17, 4, 5, 18, 6, 7, 19,
                       8, 9, 20, 10, 11, 21, 12, 13, 22, 14, 15, 23]
            qkv1 = iter(A_ORDER)

            def fill_qkv1(_h):
                for _ in range(2):
                    u = next(qkv1, None)
                    if u is not None:
                        emit_qkv_unit(1, u)

            emit_attn(0, fill_qkv1, warm=warm0)
            # warm-start seg1's pipeline the same way: its first two heads'
            # score chains begin while seg0's tail PV work runs on the PE
            warm1 = {0: (emit_scores(1, 0),
                         psO.tile([65, M], FP32, tag="psO", name="po")),
                     1: (emit_scores(1, 1),
                         psO.tile([65, M], FP32, tag="psO", name="po"))}

            h1_sb = {}

            def fill_attn1(h):
                # slots 0-7: seg0 projection tiles; slots 8-13: first halves
                # (ct 0-3) of seg1 projection tiles, whose oT deps (heads
                # 0-7) are normed by then. Slots 14-15 stay empty so the
                # final heads' norm chains aren't queued behind filler casts
                # on the vector engine.
                if h < 8:
                    emit_proj_tile(0, h)
                elif h < 14:
                    h1_sb[h - 8] = emit_proj_h1(1, h - 8)

            emit_attn(1, fill_attn1, warm=warm1)
            # tail: ot6/ot7 as full tiles (their slots had no filler), the
            # rest as second halves (ct 4-7). Every tile's ct7 needs the
            # last head's norm; borrow the score pool's four PSUM banks
            # (idle by now) plus psA so six groups cover that latency.
            # psS rotation: slots -> ps0, ps1, ps6, ps7, then ps4 (reuses
            # ps0's slot after fin0), ps5 (ps1's after fin1)
            ps_h2 = {}
            for ot in (0, 1):
                ps_h2[ot] = psS.tile([128, M], FP32, tag="psS", name="ps")
            for ot in (6, 7):
                ps_h2[ot] = psS.tile([128, M], FP32, tag="psS", name="ps")
            for ot in (2, 3):
                ps_h2[ot] = psA.tile([128, M], FP32, tag="psA", name="ps")
            for ot in (6, 7):
                for ct in range(7):
                    mm("proj1b",
                        ps_h2[ot][:],
                        lhsT=wout_sb[:, ot * 1024 + ct * 128:
                                     ot * 1024 + (ct + 1) * 128],
                        rhs=oT_sb[1][ct][:],
                        start=(ct == 0), stop=False)
            for ot in (0, 1, 2, 3):
                emit_proj_h2_mms(1, ot, (4, 5, 6), ps_h2[ot])
            for ot in (0, 1):
                emit_proj_h2_mms(1, ot, (7,), ps_h2[ot])
                emit_proj_h2_fin(1, ot, ps_h2[ot], h1_sb[ot])
                ps_h2[ot + 4] = psS.tile([128, M], FP32, tag="psS", name="ps")
                emit_proj_h2_mms(1, ot + 4, (4, 5, 6), ps_h2[ot + 4])
            for ot in (2, 3, 4, 5):
                emit_proj_h2_mms(1, ot, (7,), ps_h2[ot])
                emit_proj_h2_fin(1, ot, ps_h2[ot], h1_sb[ot])
            for ot in (6, 7):
                mm("proj1b",
                    ps_h2[ot][:],
                    lhsT=wout_sb[:, ot * 1024 + 7 * 128:ot * 1024 + 8 * 128],
                    rhs=oT_sb[1][7][:], start=False, stop=True)
                yt = ypool.tile([128, M], BF, tag="y", name="yt")
                # scalar engine is idle at the tail; vector still has the
                # ot0-5 fins queued
                nc.scalar.activation(
                    out=yt[:], in_=ps_h2[ot][:],
                    func=mybir.ActivationFunctionType.Identity,
                    bias=bout_sb[:, ot:ot + 1], scale=1.0)
                eng = nc.sync if ot % 2 == 0 else nc.scalar
                eng.dma_start(
                    out=yT[C + ot * 128:C + (ot + 1) * 128, :], in_=yt[:])

    nc.finalize()
    return nc


def _prep_inputs(x, w_in, b_in, w_out, b_out):
    x = np.asarray(x, dtype=np.float32)
    w_in = np.asarray(w_in, dtype=np.float32)
    b_in = np.asarray(b_in, dtype=np.float32)
    w_out = np.asarray(w_out, dtype=np.float32)
    b_out = np.asarray(b_out, dtype=np.float32)

    # fold 1/sqrt(dh) into the q rows of w_in / b_in
    w_in_s = w_in.copy()
    b_in_s = b_in.copy()
    w_in_s[:C] *= DH ** -0.5
    b_in_s[:C] *= DH ** -0.5

    w_inT0 = np.ascontiguousarray(w_in_s.T).astype(BF16)  # (C, 3C)

    # wqk chunks: for c4, e-tile off = pp*2 + half covers head-pair
    # p = c4*2+pp, half 0 = q, 1 = k; cols within e-tile are ct*128
    # (contraction-row-major). wqk[c4*128:(c4+1)*128, off*1024 + ct*128 + j]
    #   = w_inT0[ct*128 + (row within chunk? no...)]
    # Device consumes lhsT = wqk_sb[c4][:, off*1024+ct*128 : +128] as the
    # [128 contraction rows (= features ct*128..), 128 out cols] block for
    # out features (half*C + p*128 .. +128).
    wp = np.empty((4, 128, 4, 8, 128), dtype=BF16)  # (c4, part, off, ct, col)
    for c4 in range(4):
        for pp in range(2):
            p = c4 * 2 + pp
            for half in range(2):
                off = pp * 2 + half
                src = w_inT0[:, half * C + p * 128: half * C + (p + 1) * 128]
                # src (1024 contraction, 128 out): block ct -> rows ct*128..
                blk = src.reshape(8, 128, 128)       # (ct, part, col)
                wp[c4, :, off, :, :] = blk.transpose(1, 0, 2)
    wqk_h = np.ascontiguousarray(wp.reshape(4 * 128, 4 * 8 * 128))

    # wv: device rhs = wv_sb[:, nf*4096 + ct*512 : +512] is the [128
    # contraction (= x features ct*128..), 512 out features (nf*512..)]
    wv0 = w_inT0[:, 2 * C:]                          # (1024, 1024) c -> vfeat
    wv_blk = wv0.reshape(8, 128, 2, 512)             # (ct, part, nf, col)
    wv_h = np.ascontiguousarray(
        wv_blk.transpose(1, 2, 0, 3).reshape(128, 8 * C))

    # wout: lhsT = wout_sb[:, ot*1024 + ct*128 : +128] = [128 contraction
    # (= o features ct*128..), 128 out features ot*128..]
    w_outT = np.ascontiguousarray(w_out.T).astype(BF16)  # (C in, C out)
    wo_blk = w_outT.reshape(8, 128, 8, 128)          # (ct, part, ot, col)
    wout_h = np.ascontiguousarray(
        wo_blk.transpose(1, 2, 0, 3).reshape(128, 8 * C))

    bqk_h = np.ascontiguousarray(b_in_s[:2 * C].reshape(16, 128).T,
                                 dtype=np.float32)
    # v bias folds exactly into an effective output bias:
    #   (p @ (v + 1 b_v^T)) / denom = (p @ v)/denom + b_v
    b_out_eff = b_out + w_out @ b_in[2 * C:]
    bout_h = np.ascontiguousarray(b_out_eff.reshape(8, 128).T, dtype=np.float32)

    # dilated gather + transpose + ct-major pack: per-core (2*128, 8*M)
    xi = x.reshape(B, S, W_SEG, C)[:, :, ::RATE, :]        # (B, S, M, C)
    xiT = np.ascontiguousarray(xi.transpose(0, 1, 3, 2)).astype(BF16)  # (B,S,C,M)
    xiT = xiT.reshape(16, 8, 128, M).transpose(0, 2, 1, 3)  # (16,128,8,M)
    xiT = np.ascontiguousarray(xiT).reshape(N_CORES, SEG_PER_CORE * 128, 8 * M)

    in_maps = []
    for c in range(N_CORES):
        in_maps.append({
            "xiT": np.ascontiguousarray(xiT[c]),
            "wqk": wqk_h,
            "wv": wv_h,
            "wout": wout_h,
            "bqk": bqk_h,
            "bout": bout_h,
        })
    return in_maps


def kernel(x, w_in, b_in, w_out, b_out, _trace=False):
    if "nc" not in _CACHE:
        _CACHE["nc"] = _build()
    nc = _CACHE["nc"]

    in_maps = _prep_inputs(x, w_in, b_in, w_out, b_out)
    res = run_bass_kernel_spmd(
        nc, in_maps, core_ids=list(range(N_CORES)), trace=_trace)
    _CACHE["last_result"] = res

    out = np.zeros((B, N, C), dtype=np.float32)
    ov = out.reshape(B, S, W_SEG, C)
    for c in range(N_CORES):
        yTc = np.asarray(res.results[c]["yT"]).astype(np.float32)  # (2C, M)
        for seg in range(SEG_PER_CORE):
            gseg = c * SEG_PER_CORE + seg
            b, s = divmod(gseg, S)
            ov[b, s, ::RATE, :] = yTc[seg * C:(seg + 1) * C, :].T
    return out
